# revision 23
# baseline (speedup 1.0000x reference)
"""Trainium2 Bass kernel for nn_Interpolator: zero-stuff upsample x8 + 128-tap FIR (SAME) + x8 gain.

Polyphase formulation with 128-sample input blocks: with m indexing 128-sample
blocks of x and n in [0, 1024),
    y[1024*m + n] = sum_{k=0}^{142} A[k, m] * H[k, n]
where A[k, m] = x[128*m + k - 7] (zero-padded) and
    H[k, n] = 8 * h[7 + 8k - n]  when 0 <= 7+8k-n < 128, else 0.
K=143 splits into a K=128 main matmul (lhsT = A column block) and a K=15 fixup
matmul from the next A column (only touches n in [896, 1024)).

The A matrices are built on the HOST (numpy stride tricks) and shipped
pre-transposed: the whole per-core input is ONE contiguous [128, 4112] fp16
load (128-partition DMAs spray across all 16 SDMA engines; odd-partition
shapes land on a single engine at ~22 GB/s).  Per signal-chunk: 3 matmuls into
PSUM [128, 1024] fp32, cast-copy to fp16 SBUF (alternating scalar/vector to
split the PSUM-read work over both PSUM-capable engines), then one fully
contiguous 256 KB store on the sync ring (2 KB per partition).  y is fp16 on
device; the host casts to fp32.  8 warmup matmuls on a zeroed tile unthrottle
the PE HAM clock gate during the initial load latency.
"""

import numpy as np

import concourse.bass as bass
import concourse.tile as tile
from concourse import bacc, mybir
from concourse.bass_utils import run_bass_kernel_spmd

B = 64
N = 32768
FACTOR = 8
NOUT = N * FACTOR  # 262144
N_CORES = 8
ROWS_PER_CORE = B // N_CORES  # 8
SIGS = 2 * ROWS_PER_CORE  # 16 signals per core (real rows then imag rows)
MP = N // 128  # 256 column blocks per signal
MCOL = MP + 1  # 257 columns of A per signal (one spill column for the fixup)
NPAD2 = 7 + N + 121  # 32896
KFIX = 15

_F16 = mybir.dt.float16
_F32 = mybir.dt.float32

_NC_CACHE = {}


def _build_nc():
    nc = bacc.Bacc(
        "TRN2",
        target_bir_lowering=False,
        debug=False,
        enable_asserts=False,
        num_devices=N_CORES,
    )
    xa = nc.dram_tensor("xa", [128, SIGS * MCOL], _F16, kind="ExternalInput")
    ha = nc.dram_tensor("ha", [128, 1024], _F16, kind="ExternalInput")
    hb = nc.dram_tensor("hb", [KFIX, 128], _F16, kind="ExternalInput")
    y = nc.dram_tensor("y", [SIGS, NOUT], _F16, kind="ExternalOutput")

    with tile.TileContext(nc) as tc:
        with (
            tc.tile_pool(name="consts", bufs=1) as consts,
            tc.tile_pool(name="opool", bufs=4) as opool,
            tc.tile_pool(name="po", bufs=1, space="PSUM") as po_pool,
        ):
            # whole per-core input in one contiguous 128-partition load
            xa_sb = consts.tile([128, SIGS * MCOL], _F16)
            nc.sync.dma_start(out=xa_sb, in_=xa.ap())
            ha_sb = consts.tile([128, 1024], _F16)
            nc.scalar.dma_start(out=ha_sb, in_=ha.ap())
            hb_sb = consts.tile([KFIX, 128], _F16)
            nc.scalar.dma_start(out=hb_sb, in_=hb.ap())

            # PE warmup: the HAM clock gate (4/8 -> 8/8) releases only after a
            # gap-free ~6.5 us burst of PE activity; the pipelined real stream
            # has micro-gaps that keep resetting the activity window, so the
            # burst must fully precede real work (16 x ~420 ns cold matmuls).
            dummy = consts.tile([128, 512], _F16)
            nc.gpsimd.memset(dummy, 0)
            for _ in range(16):
                warm_po = po_pool.tile([128, 1024], _F32, tag="po")
                nc.tensor.matmul(
                    warm_po[:, 0:512], dummy[:, 0:128], dummy[:, :], start=True, stop=True
                )

            for sig in range(SIGS):
                out_sb = opool.tile([128, 2048], _F16)
                for c in range(2):
                    col = sig * MCOL + 128 * c
                    po = po_pool.tile([128, 1024], _F32, tag="po")
                    lhsT = xa_sb[0:128, col : col + 128]
                    nc.tensor.matmul(
                        po[:, 0:512], lhsT, ha_sb[:, 0:512], start=True, stop=True
                    )
                    nc.tensor.matmul(
                        po[:, 512:1024],
                        lhsT,
                        ha_sb[:, 512:1024],
                        start=True,
                        stop=False,
                    )
                    nc.tensor.matmul(
                        po[:, 896:1024],
                        xa_sb[0:KFIX, col + 1 : col + 129],
                        hb_sb[:, :],
                        start=False,
                        stop=True,
                    )
                    # the two chunks of one signal land on different engines
                    # concurrently (ACTIVATE ~1403 ns vs CAST ~1394 ns with
                    # per-op overhead -- a 16/16 split balances both streams)
                    dst = out_sb[:, 1024 * c : 1024 * (c + 1)]
                    if c == 0:
                        nc.scalar.copy(out=dst, in_=po)
                    else:
                        nc.vector.tensor_copy(out=dst, in_=po)
                # one 512 KB store in 2 KB runs: y[sig, 131072c + 1024i + j]
                nc.sync.dma_start(
                    out=bass.AP(
                        tensor=y,
                        offset=sig * NOUT,
                        ap=[[1024, 128], [131072, 2], [1, 1024]],
                    ),
                    in_=out_sb,
                )

    nc.compile()
    return nc


def _get_nc():
    if "nc" not in _NC_CACHE:
        _NC_CACHE["nc"] = _build_nc()
    return _NC_CACHE["nc"]


def _build_h(h):
    """H[k, n] = 8 h[7 + 8k - n] when 0 <= 7+8k-n < 128; returns (Ha, Hb)."""
    H = np.zeros((143, 1024), np.float32)
    k = np.arange(143)[:, None]
    n = np.arange(1024)[None, :]
    i = 7 + 8 * k - n
    m = (i >= 0) & (i < 128)
    H[m] = FACTOR * np.asarray(h, np.float32)[i[m]]
    return H[0:128].astype(np.float16), H[128:143, 896:1024].astype(np.float16)


def _run(x_real, x_imag, fir_filter, trace=False):
    ha, hb = _build_h(np.asarray(fir_filter, np.float32))
    in_maps = []
    for c in range(N_CORES):
        rows = slice(c * ROWS_PER_CORE, (c + 1) * ROWS_PER_CORE)
        xp = np.zeros((SIGS, NPAD2), np.float16)
        xp[:ROWS_PER_CORE, 7 : 7 + N] = x_real[rows]
        xp[ROWS_PER_CORE:, 7 : 7 + N] = x_imag[rows]
        # A[sig, k, m] = xp[sig, 128*m + k] -> device layout [k, sig*MCOL + m]
        v = np.lib.stride_tricks.as_strided(
            xp,
            shape=(SIGS, 128, MCOL),
            strides=(xp.strides[1] * NPAD2, xp.strides[1], 128 * xp.strides[1]),
        )
        xa = np.ascontiguousarray(np.transpose(v, (1, 0, 2))).reshape(128, SIGS * MCOL)
        in_maps.append({"xa": xa, "ha": ha, "hb": hb})
    nc = _get_nc()
    res = run_bass_kernel_spmd(nc, in_maps, core_ids=list(range(N_CORES)), trace=trace)
    out = np.empty((2, B, NOUT), np.float32)
    for c in range(N_CORES):
        yc = res.results[c]["y"]
        rows = slice(c * ROWS_PER_CORE, (c + 1) * ROWS_PER_CORE)
        out[0, rows] = yc[:ROWS_PER_CORE]
        out[1, rows] = yc[ROWS_PER_CORE:]
    return out, res


def kernel(x_real, x_imag, fir_filter, factor):
    assert int(factor) == FACTOR
    x_real = np.asarray(x_real, np.float32)
    x_imag = np.asarray(x_imag, np.float32)
    assert x_real.shape == (B, N) and x_imag.shape == (B, N)
    out, _ = _run(x_real, x_imag, fir_filter)
    return out


# revision 25
# speedup vs baseline: 2.2305x; 2.2305x over previous
"""Trainium2 Bass kernel for nn_Interpolator: zero-stuff upsample x8 + 128-tap FIR (SAME) + x8 gain.

Polyphase formulation with 128-sample input blocks: with m indexing 128-sample
blocks of x and n in [0, 1024),
    y[1024*m + n] = sum_{k=0}^{142} A[k, m] * H[k, n]
where A[k, m] = x[128*m + k - 7] (zero-padded) and
    H[k, n] = 8 * h[7 + 8k - n]  when 0 <= 7+8k-n < 128, else 0.
K=143 splits into a K=128 main matmul (lhsT = A column block) and a K=15 fixup
matmul from the next A column (only touches n in [896, 1024)).

The A matrices are built on the HOST (numpy stride tricks) and shipped
pre-transposed: the whole per-core input is ONE contiguous [128, 4112] fp16
load (128-partition DMAs spray across all 16 SDMA engines; odd-partition
shapes land on a single engine at ~22 GB/s).  Per signal-chunk: 3 matmuls into
PSUM [128, 1024] fp32, cast-copy to fp16 SBUF (alternating scalar/vector to
split the PSUM-read work over both PSUM-capable engines), then one fully
contiguous 256 KB store on the sync ring (2 KB per partition).  y is fp16 on
device; the host casts to fp32.  8 warmup matmuls on a zeroed tile unthrottle
the PE HAM clock gate during the initial load latency.
"""

import numpy as np

import concourse.bass as bass
import concourse.tile as tile
from concourse import bacc, mybir
from concourse.bass_utils import run_bass_kernel_spmd

B = 64
N = 32768
FACTOR = 8
NOUT = N * FACTOR  # 262144
N_CORES = 8
ROWS_PER_CORE = B // N_CORES  # 8
SIGS = 2 * ROWS_PER_CORE  # 16 signals per core (real rows then imag rows)
MP = N // 128  # 256 column blocks per signal
MCOL = MP + 1  # 257 columns of A per signal (one spill column for the fixup)
NPAD2 = 7 + N + 121  # 32896
KFIX = 15

_F16 = mybir.dt.float16
_F32 = mybir.dt.float32

_NC_CACHE = {}


def _build_nc():
    nc = bacc.Bacc(
        "TRN2",
        target_bir_lowering=False,
        debug=False,
        enable_asserts=False,
        num_devices=N_CORES,
    )
    xa = nc.dram_tensor("xa", [128, SIGS * MCOL], _F16, kind="ExternalInput")
    ha = nc.dram_tensor("ha", [128, 1024], _F16, kind="ExternalInput")
    hb = nc.dram_tensor("hb", [KFIX, 128], _F16, kind="ExternalInput")
    y = nc.dram_tensor("y", [SIGS, NOUT], _F16, kind="ExternalOutput")

    with tile.TileContext(nc) as tc:
        with (
            tc.tile_pool(name="consts", bufs=1) as consts,
            tc.tile_pool(name="opool", bufs=8) as opool,
            tc.tile_pool(name="po", bufs=4, space="PSUM") as po_pool,
        ):
            # whole per-core input in one contiguous 128-partition load
            xa_sb = consts.tile([128, SIGS * MCOL], _F16)
            nc.sync.dma_start(out=xa_sb, in_=xa.ap())
            ha_sb = consts.tile([128, 1024], _F16)
            nc.scalar.dma_start(out=ha_sb, in_=ha.ap())
            hb_sb = consts.tile([KFIX, 128], _F16)
            nc.scalar.dma_start(out=hb_sb, in_=hb.ap())

            # PE warmup: the HAM clock gate (4/8 -> 8/8) releases only after a
            # gap-free ~6.5 us burst of PE activity; the pipelined real stream
            # has micro-gaps that keep resetting the activity window, so the
            # burst must fully precede real work (16 x ~420 ns cold matmuls).
            dummy = consts.tile([128, 512], _F16)
            nc.gpsimd.memset(dummy, 0)
            for _ in range(16):
                warm_po = po_pool.tile([128, 1024], _F32, tag="po")
                nc.tensor.matmul(
                    warm_po[:, 0:512], dummy[:, 0:128], dummy[:, :], start=True, stop=True
                )

            for it in range(2 * SIGS):
                sig, c = it // 2, it % 2
                col = sig * MCOL + 128 * c
                po = po_pool.tile([128, 1024], _F32, tag="po")
                lhsT = xa_sb[0:128, col : col + 128]
                nc.tensor.matmul(
                    po[:, 0:512], lhsT, ha_sb[:, 0:512], start=True, stop=True
                )
                nc.tensor.matmul(
                    po[:, 512:1024], lhsT, ha_sb[:, 512:1024], start=True, stop=False
                )
                nc.tensor.matmul(
                    po[:, 896:1024],
                    xa_sb[0:KFIX, col + 1 : col + 129],
                    hb_sb[:, :],
                    start=False,
                    stop=True,
                )
                out_sb = opool.tile([128, 1024], _F16)
                # alternate engines 16/16 (ACTIVATE ~1403 ns vs CAST ~1394 ns
                # effective, including per-op overhead)
                if it % 2 == 0:
                    nc.scalar.copy(out=out_sb, in_=po)
                else:
                    nc.vector.tensor_copy(out=out_sb, in_=po)
                # fully contiguous 256 KB store: y[sig, 131072c + 1024i + j]
                nc.sync.dma_start(
                    out=bass.AP(
                        tensor=y,
                        offset=sig * NOUT + c * 131072,
                        ap=[[1024, 128], [1, 1024]],
                    ),
                    in_=out_sb,
                )

    nc.compile()
    return nc


def _get_nc():
    if "nc" not in _NC_CACHE:
        _NC_CACHE["nc"] = _build_nc()
    return _NC_CACHE["nc"]


def _build_h(h):
    """H[k, n] = 8 h[7 + 8k - n] when 0 <= 7+8k-n < 128; returns (Ha, Hb)."""
    H = np.zeros((143, 1024), np.float32)
    k = np.arange(143)[:, None]
    n = np.arange(1024)[None, :]
    i = 7 + 8 * k - n
    m = (i >= 0) & (i < 128)
    H[m] = FACTOR * np.asarray(h, np.float32)[i[m]]
    return H[0:128].astype(np.float16), H[128:143, 896:1024].astype(np.float16)


def _run(x_real, x_imag, fir_filter, trace=False):
    ha, hb = _build_h(np.asarray(fir_filter, np.float32))
    in_maps = []
    for c in range(N_CORES):
        rows = slice(c * ROWS_PER_CORE, (c + 1) * ROWS_PER_CORE)
        xp = np.zeros((SIGS, NPAD2), np.float16)
        xp[:ROWS_PER_CORE, 7 : 7 + N] = x_real[rows]
        xp[ROWS_PER_CORE:, 7 : 7 + N] = x_imag[rows]
        # A[sig, k, m] = xp[sig, 128*m + k] -> device layout [k, sig*MCOL + m]
        v = np.lib.stride_tricks.as_strided(
            xp,
            shape=(SIGS, 128, MCOL),
            strides=(xp.strides[1] * NPAD2, xp.strides[1], 128 * xp.strides[1]),
        )
        xa = np.ascontiguousarray(np.transpose(v, (1, 0, 2))).reshape(128, SIGS * MCOL)
        in_maps.append({"xa": xa, "ha": ha, "hb": hb})
    nc = _get_nc()
    res = run_bass_kernel_spmd(nc, in_maps, core_ids=list(range(N_CORES)), trace=trace)
    out = np.empty((2, B, NOUT), np.float32)
    for c in range(N_CORES):
        yc = res.results[c]["y"]
        rows = slice(c * ROWS_PER_CORE, (c + 1) * ROWS_PER_CORE)
        out[0, rows] = yc[:ROWS_PER_CORE]
        out[1, rows] = yc[ROWS_PER_CORE:]
    return out, res


def kernel(x_real, x_imag, fir_filter, factor):
    assert int(factor) == FACTOR
    x_real = np.asarray(x_real, np.float32)
    x_imag = np.asarray(x_imag, np.float32)
    assert x_real.shape == (B, N) and x_imag.shape == (B, N)
    out, _ = _run(x_real, x_imag, fir_filter)
    return out


# revision 27
# speedup vs baseline: 2.3256x; 1.0426x over previous
"""Trainium2 Bass kernel for nn_Interpolator: zero-stuff upsample x8 + 128-tap FIR (SAME) + x8 gain.

Polyphase formulation with 128-sample input blocks: with m indexing 128-sample
blocks of x and n in [0, 1024),
    y[1024*m + n] = sum_{k=0}^{142} A[k, m] * H[k, n]
where A[k, m] = x[128*m + k - 7] (zero-padded) and
    H[k, n] = 8 * h[7 + 8k - n]  when 0 <= 7+8k-n < 128, else 0.
K=143 splits into a K=128 main matmul (lhsT = A column block) and a K=15 fixup
matmul from the next A column (only touches n in [896, 1024)).

The A matrices are built on the HOST (numpy stride tricks) and shipped
pre-transposed: the whole per-core input is ONE contiguous [128, 4112] fp16
load (128-partition DMAs spray across all 16 SDMA engines; odd-partition
shapes land on a single engine at ~22 GB/s).  Per signal-chunk: 3 matmuls into
PSUM [128, 1024] fp32, cast-copy to fp16 SBUF (alternating scalar/vector to
split the PSUM-read work over both PSUM-capable engines), then one fully
contiguous 256 KB store on the sync ring (2 KB per partition).  y is fp16 on
device; the host casts to fp32.  8 warmup matmuls on a zeroed tile unthrottle
the PE HAM clock gate during the initial load latency.
"""

import numpy as np

import concourse.bass as bass
import concourse.tile as tile
from concourse import bacc, mybir
from concourse.bass_utils import run_bass_kernel_spmd

B = 64
N = 32768
FACTOR = 8
NOUT = N * FACTOR  # 262144
N_CORES = 8
ROWS_PER_CORE = B // N_CORES  # 8
SIGS = 2 * ROWS_PER_CORE  # 16 signals per core (real rows then imag rows)
MP = N // 128  # 256 column blocks per signal
MCOL = MP + 1  # 257 columns of A per signal (one spill column for the fixup)
NPAD2 = 7 + N + 121  # 32896
KFIX = 15

_F16 = mybir.dt.float16
_F32 = mybir.dt.float32

_NC_CACHE = {}


def _build_nc():
    nc = bacc.Bacc(
        "TRN2",
        target_bir_lowering=False,
        debug=False,
        enable_asserts=False,
        num_devices=N_CORES,
    )
    xa = nc.dram_tensor("xa", [128, SIGS * MCOL], _F16, kind="ExternalInput")
    ha = nc.dram_tensor("ha", [128, 1024], _F16, kind="ExternalInput")
    hb = nc.dram_tensor("hb", [KFIX, 128], _F16, kind="ExternalInput")
    y = nc.dram_tensor("y", [SIGS, NOUT], _F16, kind="ExternalOutput")

    with tile.TileContext(nc) as tc:
        with (
            tc.tile_pool(name="consts", bufs=1) as consts,
            tc.tile_pool(name="opool", bufs=8) as opool,
            tc.tile_pool(name="po", bufs=4, space="PSUM") as po_pool,
        ):
            # per-core input as four 128-partition loads (4 signals each) so the
            # first signals' data (and its completion semaphore) lands early
            GRP = 4
            xa_g = []
            for g in range(SIGS // GRP):
                t = consts.tile([128, GRP * MCOL], _F16, tag=f"xa{g}")
                nc.sync.dma_start(
                    out=t,
                    in_=bass.AP(
                        tensor=xa,
                        offset=g * GRP * MCOL,
                        ap=[[SIGS * MCOL, 128], [1, GRP * MCOL]],
                    ),
                )
                xa_g.append(t)
            ha_sb = consts.tile([128, 1024], _F16)
            nc.scalar.dma_start(out=ha_sb, in_=ha.ap())
            hb_sb = consts.tile([KFIX, 128], _F16)
            nc.scalar.dma_start(out=hb_sb, in_=hb.ap())

            # PE warmup: the HAM clock gate (4/8 -> 8/8) releases only after a
            # gap-free ~6.5 us burst of PE activity; the pipelined real stream
            # has micro-gaps that keep resetting the activity window, so the
            # burst must fully precede real work (16 x ~420 ns cold matmuls).
            dummy = consts.tile([128, 512], _F16)
            nc.gpsimd.memset(dummy, 0)
            for _ in range(16):
                warm_po = po_pool.tile([128, 1024], _F32, tag="po")
                nc.tensor.matmul(
                    warm_po[:, 0:512], dummy[:, 0:128], dummy[:, :], start=True, stop=True
                )

            for it in range(2 * SIGS):
                sig, c = it // 2, it % 2
                xs = xa_g[sig // GRP]
                col = (sig % GRP) * MCOL + 128 * c
                po = po_pool.tile([128, 1024], _F32, tag="po")
                lhsT = xs[0:128, col : col + 128]
                nc.tensor.matmul(
                    po[:, 0:512], lhsT, ha_sb[:, 0:512], start=True, stop=True
                )
                nc.tensor.matmul(
                    po[:, 512:1024], lhsT, ha_sb[:, 512:1024], start=True, stop=False
                )
                nc.tensor.matmul(
                    po[:, 896:1024],
                    xs[0:KFIX, col + 1 : col + 129],
                    hb_sb[:, :],
                    start=False,
                    stop=True,
                )
                out_sb = opool.tile([128, 1024], _F16)
                # alternate engines 16/16 (ACTIVATE ~1403 ns vs CAST ~1394 ns
                # effective, including per-op overhead); split the last copy
                # across both engines to shorten the pipeline tail
                if it == 2 * SIGS - 1:
                    nc.scalar.copy(out=out_sb[:, 0:512], in_=po[:, 0:512])
                    nc.vector.tensor_copy(out=out_sb[:, 512:1024], in_=po[:, 512:1024])
                elif it % 2 == 0:
                    nc.scalar.copy(out=out_sb, in_=po)
                else:
                    nc.vector.tensor_copy(out=out_sb, in_=po)
                # fully contiguous 256 KB store: y[sig, 131072c + 1024i + j]
                nc.sync.dma_start(
                    out=bass.AP(
                        tensor=y,
                        offset=sig * NOUT + c * 131072,
                        ap=[[1024, 128], [1, 1024]],
                    ),
                    in_=out_sb,
                )

    nc.compile()
    return nc


def _get_nc():
    if "nc" not in _NC_CACHE:
        _NC_CACHE["nc"] = _build_nc()
    return _NC_CACHE["nc"]


def _build_h(h):
    """H[k, n] = 8 h[7 + 8k - n] when 0 <= 7+8k-n < 128; returns (Ha, Hb)."""
    H = np.zeros((143, 1024), np.float32)
    k = np.arange(143)[:, None]
    n = np.arange(1024)[None, :]
    i = 7 + 8 * k - n
    m = (i >= 0) & (i < 128)
    H[m] = FACTOR * np.asarray(h, np.float32)[i[m]]
    return H[0:128].astype(np.float16), H[128:143, 896:1024].astype(np.float16)


def _run(x_real, x_imag, fir_filter, trace=False):
    ha, hb = _build_h(np.asarray(fir_filter, np.float32))
    in_maps = []
    for c in range(N_CORES):
        rows = slice(c * ROWS_PER_CORE, (c + 1) * ROWS_PER_CORE)
        xp = np.zeros((SIGS, NPAD2), np.float16)
        xp[:ROWS_PER_CORE, 7 : 7 + N] = x_real[rows]
        xp[ROWS_PER_CORE:, 7 : 7 + N] = x_imag[rows]
        # A[sig, k, m] = xp[sig, 128*m + k] -> device layout [k, sig*MCOL + m]
        v = np.lib.stride_tricks.as_strided(
            xp,
            shape=(SIGS, 128, MCOL),
            strides=(xp.strides[1] * NPAD2, xp.strides[1], 128 * xp.strides[1]),
        )
        xa = np.ascontiguousarray(np.transpose(v, (1, 0, 2))).reshape(128, SIGS * MCOL)
        in_maps.append({"xa": xa, "ha": ha, "hb": hb})
    nc = _get_nc()
    res = run_bass_kernel_spmd(nc, in_maps, core_ids=list(range(N_CORES)), trace=trace)
    out = np.empty((2, B, NOUT), np.float32)
    for c in range(N_CORES):
        yc = res.results[c]["y"]
        rows = slice(c * ROWS_PER_CORE, (c + 1) * ROWS_PER_CORE)
        out[0, rows] = yc[:ROWS_PER_CORE]
        out[1, rows] = yc[ROWS_PER_CORE:]
    return out, res


def kernel(x_real, x_imag, fir_filter, factor):
    assert int(factor) == FACTOR
    x_real = np.asarray(x_real, np.float32)
    x_imag = np.asarray(x_imag, np.float32)
    assert x_real.shape == (B, N) and x_imag.shape == (B, N)
    out, _ = _run(x_real, x_imag, fir_filter)
    return out
